# revision 1
# baseline (speedup 1.0000x reference)
"""Distributed 3-layer GCN (AqSolModel) on 8 TRN2 NeuronCores.

Strategy
--------
Nodes are partitioned by graph id (2048 graphs -> 256 graphs/core, nodes of a
graph never cross cores, so the segment-mean pool is core-local).  Per layer:

  z = (h @ W) scaled per-row by G_l*dis (dis=1/sqrt(deg); G_l is a per-layer
  gain that keeps fp8 values in normal range), stored as two fp8 tables
  (node slots 0-63 / 64-127 of each tile) so table row ids fit int16;
  AllGather both tables across the 8 cores; per dst-tile of 128 nodes,
  dma_gather (custom Q7 SWDGE instruction, <=1024 rows per call to fit the
  descriptor ring) fetches the tile's in-edge source rows, PE segment-sums
  them with one-hot selection matmuls (sel built on DVE via iota/is_equal),
  the self-loop term is added as an identity matmul from the local z store,
  and h = relu(dis/G_l * agg + b) is one ACT op (bias folded in as a K=1
  outer-product matmul with the sqrt(deg) row, so GCN's symmetric norm
  comes out exactly).  The segment-mean pool + MLP head run per-core in a
  transposed layout (graphs never cross cores).

The per-edge gather (3 x ~30 MB/core of 256B fp8 rows) is the memory-bound
core of the problem; DVE/PE/ACT work overlaps under it.  Measured ~3.6 ms
on 8 NeuronCores at rel err 2.9e-3.
"""

import os
import sys
import numpy as np

sys.path.insert(0, "/opt/trn_rl_repo")

import ml_dtypes

import concourse.bass as bass
import concourse.bacc as bacc
import concourse.mybir as mybir
import concourse.tile as tile
from concourse.masks import make_identity

N_NODES = 50000
N_EDGES = 800000
N_GRAPHS = 2048
N_FEAT = 64
HIDDEN = 256
N_CORES = 8
GPC = N_GRAPHS // N_CORES          # graphs per core (256)
GPW = GPC // 2                     # graphs per window (128)

F32 = mybir.dt.float32
BF16 = mybir.dt.bfloat16
FP8 = mybir.dt.float8e4
GDT = FP8            # gather-table dtype (z table, gathered rows, sel)
GAINS = (64.0, 1024.0, 8192.0)   # per-layer fp8 dynamic-range gains
I32 = mybir.dt.int32
I16 = mybir.dt.int16
BF16_NP = ml_dtypes.bfloat16


# ---------------------------------------------------------------- host side --

def preprocess(x, edge_index, batch):
    """Shard the graph across cores.  Returns (cfg, per-core input arrays)."""
    x = np.asarray(x, np.float32)
    src_g = np.asarray(edge_index[0], np.int64)
    dst_g = np.asarray(edge_index[1], np.int64)
    batch = np.asarray(batch, np.int64)

    # node -> core / half-window, contiguous because batch is sorted
    gsplit = np.searchsorted(batch, np.arange(0, N_GRAPHS + 1, GPW))  # 17 cuts
    half_cnt = np.diff(gsplit)                       # nodes per (core, half)
    T_half = int(np.max((half_cnt + 127) // 128))
    T_tiles = 2 * T_half
    n_c = T_tiles * 128                              # node slots per core

    # slot of each node inside its core
    core_of = np.repeat(np.arange(16) // 2, half_cnt)           # per node
    half_of = np.repeat(np.arange(16) % 2, half_cnt)
    rank_in_half = np.arange(N_NODES) - np.repeat(gsplit[:-1], half_cnt)
    slot = half_of * (T_half * 128) + rank_in_half
    grow = core_of * n_c + slot                                  # global row id

    # degree (in-degree + self loop) and norm factors
    deg = np.bincount(dst_g, minlength=N_NODES).astype(np.float64) + 1.0
    dis = (1.0 / np.sqrt(deg)).astype(np.float32)
    invdis = np.sqrt(deg).astype(np.float32)

    # real edges only; the self-loop term (z[v] into agg[v]) is applied on
    # device as an identity matmul from the locally stored z tile
    e_src = src_g
    e_dst = dst_g
    e_core = core_of[e_dst]
    e_tile = slot[e_dst] // 128
    e_local = slot[e_dst] % 128

    # src half: q<64 -> table E, q>=64 -> table O; table idx within core
    e_sq = slot[e_src]
    e_half = ((e_sq % 128) >= 64).astype(np.int64)               # 0=E, 1=O
    e_tabidx = (core_of[e_src] * (n_c // 2) + (e_sq // 128) * 64
                + (e_sq % 64)).astype(np.int64)

    # sort edges by (core, tile, src half, table idx) -- src order for DMA
    order = np.lexsort((e_tabidx, e_half, e_tile, e_core))
    e_core, e_tile, e_local, e_half, e_tabidx = (
        a[order] for a in (e_core, e_tile, e_local, e_half, e_tabidx))

    counts = np.zeros((N_CORES, T_tiles, 2), np.int64)
    np.add.at(counts, (e_core, e_tile, e_half), 1)
    ghalf = ((counts.max(axis=0) + 127) // 128).astype(np.int64)  # [T,2]
    kt = (ghalf[:, 0] + ghalf[:, 1]).astype(np.int64)             # K-tiles/t
    koff = np.concatenate([[0], np.cumsum(kt)])
    t_kt = int(koff[-1])

    # pack edst [128, t_kt] and wrapped int16 gather indices [128, t_kt*8]
    edst = np.full((N_CORES, 128, t_kt), -1, np.float32)
    gidx = np.zeros((N_CORES, 16, t_kt * 8), np.int16)
    flat_bucket = (e_core * T_tiles + e_tile) * 2 + e_half
    bb = np.zeros(N_CORES * T_tiles * 2 + 1, np.int64)
    np.add.at(bb, flat_bucket + 1, 1)
    bb = np.cumsum(bb)
    pos_in_bucket = np.arange(len(e_tabidx)) - bb[flat_bucket]
    # K-tile column of this edge: tile base + half offset + within-half tile
    halfbase = koff[e_tile] + e_half * ghalf[e_tile, 0]
    col = halfbase + pos_in_bucket // 128
    p_idx = pos_in_bucket % 128
    edst[e_core, p_idx, col] = e_local.astype(np.float32)
    # wrapped idx layout: edge i of its gather -> [i%16, gather_off*8 + i//16]
    # i = (pos within the half bucket); gather off in columns-of-8 = halfbase*8
    i_in_g = pos_in_bucket
    gidx[e_core, i_in_g % 16, halfbase * 8 + i_in_g // 16] = \
        e_tabidx.astype(np.int16)

    # per-core node-side arrays
    xT = np.zeros((N_CORES, N_FEAT, n_c), BF16_NP)
    dis_a = np.zeros((N_CORES, 128, T_tiles), np.float32)
    invdis_a = np.zeros((N_CORES, 1, n_c), BF16_NP)
    batchloc = np.full((N_CORES, 128, T_tiles), -1, np.float32)
    xT[core_of, :, slot] = x.astype(BF16_NP)        # fancy: (N, F) -> rows
    dis_a[core_of, slot % 128, slot // 128] = dis
    invdis_a[core_of, 0, slot] = invdis.astype(BF16_NP)
    batchloc[core_of, slot % 128, slot // 128] = (batch % GPW).astype(np.float32)

    # per-graph 1/max(count,1), [1, 256] per core (window-major)
    gcnt = np.bincount(batch, minlength=N_GRAPHS).astype(np.float32)
    rec = (1.0 / np.maximum(gcnt, 1.0)).reshape(N_CORES, 1, GPC)
    rec = np.ascontiguousarray(np.broadcast_to(rec, (N_CORES, 128, GPC)))

    # per-layer fp8 gain: z stored as G_l * z, undone in the epilogue
    disg = np.concatenate([dis_a * g for g in GAINS], axis=2)     # z-store
    disu = np.concatenate([dis_a / g for g in GAINS], axis=2)     # epilogue
    # per-(tile, half, chunk-of-8-Ktiles) valid-row counts, build order
    # must match the device loop exactly
    bucket_cnt = counts            # [N_CORES, T_tiles, 2]
    gcounts = []
    for t in range(T_tiles):
        for h in range(2):
            gn = int(ghalf[t, h])
            for g0 in range(0, gn, 8):
                gc_ = min(8, gn - g0)
                cval = np.clip(bucket_cnt[:, t, h] - g0 * 128, 0, gc_ * 128)
                gcounts.append(cval)
    gcounts = np.stack(gcounts, axis=1).astype(np.int32)   # [N_CORES, n_g]
    # every chunk needs >=1 valid idx (ucode/sim quirk); point it at row 0
    koff8 = []
    for t in range(T_tiles):
        for h in range(2):
            gn = int(ghalf[t, h])
            base = int(koff[t]) + (h * int(ghalf[t, 0]))
            for g0 in range(0, gn, 8):
                koff8.append(base + g0)
    for c in range(N_CORES):
        for g, cbase in enumerate(koff8):
            if gcounts[c, g] == 0:
                gidx[c, 0, cbase * 8] = 0
                gcounts[c, g] = 1
    tail = (counts.min(axis=0) // 128).astype(np.int64)       # [T,2]
    gmax = counts.max(axis=0).astype(np.int64)                 # [T,2]
    cfg = dict(T_half=T_half, kt=tuple(int(v) for v in kt),
               ghalf=tuple((int(a), int(b)) for a, b in ghalf),
               tail=tuple((int(a), int(b)) for a, b in tail),
               gmax=tuple((int(a), int(b)) for a, b in gmax))
    gidx = np.tile(gidx, (1, 8, 1)) if gidx.shape[1] == 16 else gidx
    arrays = dict(xT=xT, disg=disg, disu=disu, invdis=invdis_a,
                  batchloc=batchloc, gidx=gidx, edst=edst, rec=rec)
    return cfg, arrays


def pack_weights(W1, b1, W2, b2, W3, b3, Wl, bl, Wo, bo):
    """Pack the (replicated) weights into device layouts."""
    def to16(a):
        return np.asarray(a, np.float32).astype(BF16_NP)

    w1 = np.zeros((128, HIDDEN), BF16_NP)
    w1[:N_FEAT] = to16(W1)                                     # [64,256] pad K
    w2 = np.asarray(W2, np.float32).reshape(2, 128, HIDDEN)
    w2 = np.concatenate([to16(w2[0]), to16(w2[1])], axis=1)    # [128, 512]
    w3 = np.asarray(W3, np.float32).reshape(2, 128, HIDDEN)
    w3 = np.concatenate([to16(w3[0]), to16(w3[1])], axis=1)
    wl = np.asarray(Wl, np.float32).reshape(2, 128, 128)
    wl = np.concatenate([to16(wl[0]), to16(wl[1])], axis=1)    # [128, 256]
    wo = to16(np.asarray(Wo, np.float32).reshape(128, 1))      # [128, 1]
    b123 = np.stack([np.asarray(b, np.float32) * g
                     for b, g in zip((b1, b2, b3), GAINS)])
    b123 = b123.astype(BF16_NP).reshape(1, 3 * HIDDEN)         # [1, 768]
    bl_a = np.asarray(bl, np.float32).reshape(128, 1)          # [128, 1]
    bo_a = np.asarray(bo, np.float32).reshape(1, 1)
    return dict(w1=w1, w2=w2, w3=w3, wl=wl, wo=wo, b123=b123,
                bl=bl_a, bo=bo_a)


# -------------------------------------------------------------- bass kernel --

def build(cfg, debug_taps=False):
    """Build the SPMD Bass program (same graph on all 8 cores)."""
    T_half = cfg["T_half"]
    kt = cfg["kt"]
    ghalf = cfg["ghalf"]
    tail = cfg["tail"]
    gmax = cfg["gmax"]
    T_tiles = 2 * T_half
    n_c = T_tiles * 128
    t_kt = int(sum(kt))
    n_rows = N_CORES * n_c
    nh_rows = n_rows // 2

    nc = bacc.Bacc(None, target_bir_lowering=False,
                   dynamic_dma_scratch_size=49152)
    P = {}
    P["xT"] = nc.declare_dram_parameter("xT", [N_FEAT, n_c], BF16, False)
    P["disg"] = nc.declare_dram_parameter("disg", [128, 3 * T_tiles], F32, False)
    P["disu"] = nc.declare_dram_parameter("disu", [128, 3 * T_tiles], F32, False)
    P["invdis"] = nc.declare_dram_parameter("invdis", [1, n_c], BF16, False)
    P["batchloc"] = nc.declare_dram_parameter("batchloc", [128, T_tiles], F32, False)
    P["gidx"] = nc.declare_dram_parameter("gidx", [128, t_kt * 8], I16, False)
    P["edst"] = nc.declare_dram_parameter("edst", [128, t_kt], F32, False)
    P["rec"] = nc.declare_dram_parameter("rec", [128, GPC], F32, False)
    P["w1"] = nc.declare_dram_parameter("w1", [128, HIDDEN], BF16, False)
    P["w2"] = nc.declare_dram_parameter("w2", [128, 2 * HIDDEN], BF16, False)
    P["w3"] = nc.declare_dram_parameter("w3", [128, 2 * HIDDEN], BF16, False)
    P["wl"] = nc.declare_dram_parameter("wl", [128, 256], BF16, False)
    P["wo"] = nc.declare_dram_parameter("wo", [128, 1], BF16, False)
    P["b123"] = nc.declare_dram_parameter("b123", [1, 3 * HIDDEN], BF16, False)
    P["bl"] = nc.declare_dram_parameter("bl", [128, 1], F32, False)
    P["bo"] = nc.declare_dram_parameter("bo", [1, 1], F32, False)
    out_ext = nc.declare_dram_parameter("out", [1, GPC], F32, True)
    if debug_taps:
        dbg_z = nc.declare_dram_parameter("dbg_z", [n_c, HIDDEN], GDT, True)
        dbg_g = nc.declare_dram_parameter("dbg_g", [128, kt[0] * HIDDEN], GDT, True)
        dbg_h = nc.declare_dram_parameter("dbg_h", [128, HIDDEN], BF16, True)
        dbg_sel = nc.declare_dram_parameter("dbg_sel", [128, 128], GDT, True)

    rg = [list(range(N_CORES))]
    AF = mybir.ActivationFunctionType
    OP = mybir.AluOpType

    with tile.TileContext(nc) as tc:
        with (
            tc.tile_pool(name="const", bufs=1) as cpool,
            tc.tile_pool(name="big", bufs=1) as bigpool,
            tc.tile_pool(name="work", bufs=3) as wpool,
            tc.tile_pool(name="gath", bufs=3) as gpool,
            tc.tile_pool(name="sel", bufs=4) as spool,
            tc.tile_pool(name="ps", bufs=2, space="PSUM") as pspool,
            tc.tile_pool(name="psa", bufs=2, space="PSUM") as papool,
            tc.tile_pool(name="pspool", bufs=1, space="PSUM") as ppool,
            tc.tile_pool(name="dram", bufs=2, space="DRAM") as dpool,
        ):
            # ---- constants / parameter loads (once) ----
            iota_t = cpool.tile([128, 128], F32)
            nc.gpsimd.iota(iota_t[:], pattern=[[1, 128]], base=0,
                           channel_multiplier=0,
                           allow_small_or_imprecise_dtypes=True)
            ident = cpool.tile([128, 128], BF16)
            make_identity(nc, ident[:])
            ident8 = cpool.tile([128, 128], GDT)
            nc.vector.tensor_copy(ident8[:], ident[:])

            def load(name, shape, dt):
                t = cpool.tile(list(shape), dt, name=f"sb_{name}")
                nc.sync.dma_start(out=t[:], in_=P[name][:, :])
                return t

            xT_sb = bigpool.tile([N_FEAT, n_c], BF16)
            nc.sync.dma_start(out=xT_sb[:], in_=P["xT"][:, :])
            disg_sb = load("disg", (128, 3 * T_tiles), F32)
            disu_sb = load("disu", (128, 3 * T_tiles), F32)
            invdis_sb = load("invdis", (1, n_c), BF16)
            batchloc_sb = load("batchloc", (128, T_tiles), F32)
            gidx_sb = bigpool.tile([128, t_kt * 8], I16)
            nc.sync.dma_start(out=gidx_sb[:], in_=P["gidx"][:, :])
            gather_no = [0]
            edst_sb = bigpool.tile([128, t_kt], F32)
            nc.sync.dma_start(out=edst_sb[:], in_=P["edst"][:, :])
            rec_sb = load("rec", (128, GPC), F32)
            w1_sb = load("w1", (128, HIDDEN), BF16)
            w2_sb = load("w2", (128, 2 * HIDDEN), BF16)
            w3_sb = load("w3", (128, 2 * HIDDEN), BF16)
            wl_sb = load("wl", (128, 256), BF16)
            wo_sb = load("wo", (128, 1), BF16)
            b123_sb = load("b123", (1, 3 * HIDDEN), BF16)
            bl_sb = load("bl", (128, 1), F32)
            bo_sb = load("bo", (1, 1), F32)

            # persistent transposed activations for the dense matmuls
            hT0 = bigpool.tile([128, n_c], BF16)
            hT1 = bigpool.tile([128, n_c], BF16)

            pool_ps = [None, None]      # [chunk] psum tiles for poolT, per win
            out_sb = cpool.tile([1, GPC], F32)

            for layer in range(3):
                w_sb = (w1_sb, w2_sb, w3_sb)[layer]
                # -------- dense: z = dis * (h @ W), bf16, to DRAM shard ------
                zloc_e = dpool.tile([n_c // 2, HIDDEN], GDT, tag="zloc_e")
                zloc_o = dpool.tile([n_c // 2, HIDDEN], GDT, tag="zloc_o")
                for t in range(T_tiles):
                    sl = slice(t * 128, (t + 1) * 128)
                    psz = pspool.tile([128, HIDDEN], F32, tag="mm")
                    if layer == 0:
                        nc.tensor.matmul(psz[:], lhsT=xT_sb[:, sl],
                                         rhs=w_sb[:N_FEAT, :HIDDEN],
                                         start=True, stop=True)
                    else:
                        nc.tensor.matmul(psz[:], lhsT=hT0[:, sl],
                                         rhs=w_sb[:, 0:HIDDEN],
                                         start=True, stop=False)
                        nc.tensor.matmul(psz[:], lhsT=hT1[:, sl],
                                         rhs=w_sb[:, HIDDEN:2 * HIDDEN],
                                         start=False, stop=True)
                    zs = wpool.tile([128, HIDDEN], GDT, tag="zs")
                    nc.scalar.activation(
                        zs[:], psz[:], AF.Copy,
                        scale=disg_sb[:, layer * T_tiles + t:
                                      layer * T_tiles + t + 1])
                    hsl = slice(t * 64, (t + 1) * 64)
                    nc.sync.dma_start(out=zloc_e[hsl, :], in_=zs[0:64, :])
                    nc.sync.dma_start(out=zloc_o[hsl, :], in_=zs[64:128, :])

                # -------- AllGather the scaled z table ----------------------
                zfull_e = dpool.tile([nh_rows, HIDDEN], GDT, tag="zfull_e",
                                     addr_space="Shared")
                zfull_o = dpool.tile([nh_rows, HIDDEN], GDT, tag="zfull_o",
                                     addr_space="Shared")
                nc.gpsimd.collective_compute(
                    "AllGather", OP.bypass, replica_groups=rg,
                    ins=[zloc_e.opt()], outs=[zfull_e.opt()])
                nc.gpsimd.collective_compute(
                    "AllGather", OP.bypass, replica_groups=rg,
                    ins=[zloc_o.opt()], outs=[zfull_o.opt()])
                if debug_taps and layer == 0:
                    nc.gpsimd.dma_start(out=dbg_z[:n_c // 2, :],
                                        in_=zloc_e[:, :])
                    nc.gpsimd.dma_start(out=dbg_z[n_c // 2:, :],
                                        in_=zloc_o[:, :])

                # -------- per dst-tile: gather + one-hot segsum -------------
                gather_no[0] = 0
                b_row = b123_sb[0:1, layer * HIDDEN:(layer + 1) * HIDDEN]
                for t in range(T_tiles):
                    sl = slice(t * 128, (t + 1) * 128)
                    ktt = kt[t]
                    ko = int(sum(kt[:t]))
                    psa = papool.tile([128, HIDDEN], F32, tag="psa")
                    # bias as outer product (1/dis)[dst] x b  -> exact norm
                    nc.tensor.matmul(
                        psa[:], lhsT=invdis_sb[0:1, sl], rhs=b_row,
                        start=True, stop=False)
                    # self-loop term: agg[v] += z[v], from the local z store
                    hsl2 = slice(t * 64, (t + 1) * 64)
                    zsl = wpool.tile([128, HIDDEN], GDT, tag="zsl")
                    nc.sync.dma_start(out=zsl[0:64, :], in_=zloc_e[hsl2, :])
                    nc.sync.dma_start(out=zsl[64:128, :], in_=zloc_o[hsl2, :])
                    nc.tensor.matmul(
                        psa[:], lhsT=ident8[:], rhs=zsl[:],
                        start=False, stop=(ktt == 0))
                    if ktt > 0:
                        ge, go = ghalf[t]
                        gath = gpool.tile([128, ktt * HIDDEN], GDT,
                                          tag="gath")
                        for (gn, tab, c0, gm) in (
                                (ge, zfull_e, 0, gmax[t][0]),
                                (go, zfull_o, ge, gmax[t][1])):
                            if gn > 0 and gm < gn * 128:
                                # rows past the max count stay unwritten in
                                # the last K-tile; zero them so sel=0 never
                                # multiplies NaN bit patterns
                                nc.vector.memset(
                                    gath[:, (c0 + gn - 1) * HIDDEN:
                                         (c0 + gn) * HIDDEN], 0.0)
                            # <=8 K-tiles (1024 rows) per instruction so one
                            # gather's descriptors fit the SWDGE ring; the
                            # last chunk gathers only the max-over-cores
                            # row count instead of the 128-rounded count
                            for g0 in range(0, gn, 8):
                                gc = min(8, gn - g0)
                                nidx = min(gc * 128, int(gm) - g0 * 128)
                                cc = c0 + g0
                                b8 = (ko + cc) * 8
                                gv = gath[:, cc * HIDDEN:
                                          (cc + gc) * HIDDEN] \
                                    .rearrange("p (k h) -> p k h", h=HIDDEN)
                                nc.gpsimd.dma_gather(
                                    out_ap=gv, in_ap=tab[:, :],
                                    idxs_ap=gidx_sb[:, b8:
                                                    b8 + (nidx + 15) // 16],
                                    num_idxs=nidx, num_idxs_reg=nidx,
                                    elem_size=HIDDEN)
                        if debug_taps and layer == 0 and t == 0:
                            nc.gpsimd.dma_start(out=dbg_g[:, :], in_=gath[:])
                        for j in range(ktt):
                            sel = spool.tile([128, 128], GDT, tag="sel")
                            nc.vector.tensor_tensor(
                                sel[:],
                                edst_sb[:, ko + j:ko + j + 1]
                                .to_broadcast([128, 128]),
                                iota_t[:], op=OP.is_equal)
                            if debug_taps and layer == 0 and t == 0 and j == 0:
                                nc.sync.dma_start(out=dbg_sel[:, :], in_=sel[:])
                            nc.tensor.matmul(
                                psa[:], lhsT=sel[:],
                                rhs=gath[:, j * HIDDEN:(j + 1) * HIDDEN],
                                start=False, stop=(j == ktt - 1))
                    # h = relu(dis * agg + b)
                    h_sb = wpool.tile([128, HIDDEN], BF16, tag="h")
                    nc.scalar.activation(
                        h_sb[:], psa[:], AF.Relu,
                        scale=disu_sb[:, layer * T_tiles + t:
                                      layer * T_tiles + t + 1])
                    if debug_taps and layer == 0 and t == 0:
                        nc.sync.dma_start(out=dbg_h[:, :], in_=h_sb[:])
                    if layer < 2:
                        for cch in range(2):
                            pst = pspool.tile([128, 128], BF16, tag="mm")
                            nc.tensor.transpose(
                                pst[:], h_sb[:, cch * 128:(cch + 1) * 128],
                                ident[:])
                            hT = (hT0, hT1)[cch]
                            nc.vector.tensor_copy(hT[:, sl], pst[:])
                    else:
                        win = t // T_half
                        first = (t % T_half) == 0
                        last = (t % T_half) == T_half - 1
                        if first:
                            pool_ps[0] = ppool.tile([128, 128], F32, name="poolT0",
                                                    tag="poolT0", bufs=1)
                            pool_ps[1] = ppool.tile([128, 128], F32, name="poolT1",
                                                    tag="poolT1", bufs=1)
                        selp = spool.tile([128, 128], BF16, tag="selp")
                        nc.vector.tensor_tensor(
                            selp[:],
                            batchloc_sb[:, t:t + 1].to_broadcast([128, 128]),
                            iota_t[:], op=OP.is_equal)
                        for cch in range(2):
                            nc.tensor.matmul(
                                pool_ps[cch][:],
                                lhsT=h_sb[:, cch * 128:(cch + 1) * 128],
                                rhs=selp[:], start=first, stop=last)
                        if last:
                            # ---- head for this window of 128 graphs ----
                            rrow = rec_sb[:, win * GPW:(win + 1) * GPW]
                            psu = papool.tile([128, GPW], F32, tag="head", bufs=1)
                            for cch in range(2):
                                gT = wpool.tile([128, GPW], BF16,
                                                tag="gT")
                                nc.vector.tensor_tensor(
                                    gT[:], pool_ps[cch][:, :GPW],
                                    rrow, op=OP.mult)
                                nc.tensor.matmul(
                                    psu[:],
                                    lhsT=wl_sb[:, cch * 128:(cch + 1) * 128],
                                    rhs=gT[:], start=(cch == 0),
                                    stop=(cch == 1))
                            uT = wpool.tile([128, GPW], BF16, tag="uT")
                            nc.scalar.activation(uT[:], psu[:], AF.Relu,
                                                 bias=bl_sb[:, 0:1])
                            pso = papool.tile([1, GPW], F32, tag="head", bufs=1)
                            nc.tensor.matmul(pso[:], lhsT=wo_sb[:, 0:1],
                                             rhs=uT[:], start=True, stop=True)
                            nc.vector.tensor_scalar(
                                out_sb[0:1, win * GPW:(win + 1) * GPW],
                                pso[:], bo_sb[0:1, 0:1], None, op0=OP.add)
            nc.sync.dma_start(out=out_ext[:, :], in_=out_sb[:])
    nc.finalize()
    return nc


# ------------------------------------------------------------------ runner --

_CACHE = {}


def _get_program(cfg):
    key = (cfg["T_half"], cfg["kt"])
    if key not in _CACHE:
        _CACHE[key] = build(cfg)
    return _CACHE[key]


def kernel(x, edge_index, batch, W1, b1, W2, b2, W3, b3, Wl, bl, Wo, bo):
    from concourse.bass_utils import run_bass_kernel_spmd

    cfg, arrays = preprocess(x, edge_index, batch)
    wts = pack_weights(W1, b1, W2, b2, W3, b3, Wl, bl, Wo, bo)
    nc = _get_program(cfg)

    in_maps = []
    for c in range(N_CORES):
        m = {k: np.ascontiguousarray(v[c]) for k, v in arrays.items()}
        m.update(wts)
        in_maps.append(m)

    res = run_bass_kernel_spmd(nc, in_maps, core_ids=list(range(N_CORES)))
    outs = res.results
    out = np.concatenate([outs[c]["out"].reshape(GPC) for c in range(N_CORES)])
    return out.reshape(N_GRAPHS, 1).astype(np.float32)



# revision 3
# speedup vs baseline: 1.7842x; 1.7842x over previous
"""Distributed 3-layer GCN (AqSolModel) on 8 TRN2 NeuronCores.

Strategy
--------
Nodes are partitioned by graph id (2048 graphs -> 256 graphs/core, nodes of a
graph never cross cores, so the segment-mean pool is core-local).  Per layer:

  z = (h @ W) scaled per-row by G_l*dis (dis=1/sqrt(deg); G_l is a per-layer
  gain that keeps fp8 values in normal range), stored as two fp8 tables
  (node slots 0-63 / 64-127 of each tile) so table row ids fit int16;
  AllGather both tables across the 8 cores; per dst-tile of 128 nodes,
  dma_gather (custom Q7 SWDGE instruction, <=1024 rows per call to fit the
  descriptor ring) fetches the tile's in-edge source rows, PE segment-sums
  them with one-hot selection matmuls (sel built on DVE via iota/is_equal),
  the self-loop term is added as an identity matmul from the local z store,
  and h = relu(dis/G_l * agg + b) is one ACT op (bias folded in as a K=1
  outer-product matmul with the sqrt(deg) row, so GCN's symmetric norm
  comes out exactly).  The segment-mean pool + MLP head run per-core in a
  transposed layout (graphs never cross cores).

The per-edge gather (3 x ~30 MB/core of 256B fp8 rows) is the memory-bound
core of the problem; DVE/PE/ACT work overlaps under it.  Measured ~3.6 ms
on 8 NeuronCores at rel err 2.9e-3.
"""

import os
import sys
import numpy as np

sys.path.insert(0, "/opt/trn_rl_repo")

import ml_dtypes

import concourse.bass as bass
import concourse.bacc as bacc
import concourse.mybir as mybir
import concourse.tile as tile
from concourse.masks import make_identity

N_NODES = 50000
N_EDGES = 800000
N_GRAPHS = 2048
N_FEAT = 64
HIDDEN = 256
N_CORES = 8
GPC = N_GRAPHS // N_CORES          # graphs per core (256)
GPW = GPC // 2                     # graphs per window (128)

F32 = mybir.dt.float32
BF16 = mybir.dt.bfloat16
FP8 = mybir.dt.float8e4
GDT = FP8            # gather-table dtype (z table, gathered rows, sel)
GAINS = (64.0, 1024.0, 8192.0)   # per-layer fp8 dynamic-range gains
I32 = mybir.dt.int32
I16 = mybir.dt.int16
BF16_NP = ml_dtypes.bfloat16


# ---------------------------------------------------------------- host side --

def preprocess(x, edge_index, batch):
    """Shard the graph across cores.  Returns (cfg, per-core input arrays)."""
    x = np.asarray(x, np.float32)
    src_g = np.asarray(edge_index[0], np.int64)
    dst_g = np.asarray(edge_index[1], np.int64)
    batch = np.asarray(batch, np.int64)

    # node -> core / half-window, contiguous because batch is sorted
    gsplit = np.searchsorted(batch, np.arange(0, N_GRAPHS + 1, GPW))  # 17 cuts
    half_cnt = np.diff(gsplit)                       # nodes per (core, half)
    T_half = int(np.max((half_cnt + 127) // 128))
    T_tiles = 2 * T_half
    n_c = T_tiles * 128                              # node slots per core

    # slot of each node inside its core
    core_of = np.repeat(np.arange(16) // 2, half_cnt)           # per node
    half_of = np.repeat(np.arange(16) % 2, half_cnt)
    rank_in_half = np.arange(N_NODES) - np.repeat(gsplit[:-1], half_cnt)
    slot = half_of * (T_half * 128) + rank_in_half
    grow = core_of * n_c + slot                                  # global row id

    # degree (in-degree + self loop) and norm factors
    deg = np.bincount(dst_g, minlength=N_NODES).astype(np.float64) + 1.0
    dis = (1.0 / np.sqrt(deg)).astype(np.float32)
    invdis = np.sqrt(deg).astype(np.float32)

    # real edges only; the self-loop term (z[v] into agg[v]) is applied on
    # device as an identity matmul from the locally stored z tile
    e_src = src_g
    e_dst = dst_g
    e_core = core_of[e_dst]
    e_tile = slot[e_dst] // 128
    e_local = slot[e_dst] % 128

    # src half: q<64 -> table E, q>=64 -> table O; table idx within core
    e_sq = slot[e_src]
    e_half = ((e_sq % 128) >= 64).astype(np.int64)               # 0=E, 1=O
    e_tabidx = (core_of[e_src] * (n_c // 2) + (e_sq // 128) * 64
                + (e_sq % 64)).astype(np.int64)

    # sort edges by (core, tile, src half, table idx) -- src order for DMA
    order = np.lexsort((e_tabidx, e_half, e_tile, e_core))
    e_core, e_tile, e_local, e_half, e_tabidx = (
        a[order] for a in (e_core, e_tile, e_local, e_half, e_tabidx))

    counts = np.zeros((N_CORES, T_tiles, 2), np.int64)
    np.add.at(counts, (e_core, e_tile, e_half), 1)
    ghalf = ((counts.max(axis=0) + 127) // 128).astype(np.int64)  # [T,2]
    kt = (ghalf[:, 0] + ghalf[:, 1]).astype(np.int64)             # K-tiles/t
    koff = np.concatenate([[0], np.cumsum(kt)])
    t_kt = int(koff[-1])

    # pack edst [128, t_kt] and wrapped int16 gather indices [128, t_kt*8]
    edst = np.full((N_CORES, 128, t_kt), -1, np.float32)
    gidx = np.zeros((N_CORES, 16, t_kt * 8), np.int16)
    flat_bucket = (e_core * T_tiles + e_tile) * 2 + e_half
    bb = np.zeros(N_CORES * T_tiles * 2 + 1, np.int64)
    np.add.at(bb, flat_bucket + 1, 1)
    bb = np.cumsum(bb)
    pos_in_bucket = np.arange(len(e_tabidx)) - bb[flat_bucket]
    # K-tile column of this edge: tile base + half offset + within-half tile
    halfbase = koff[e_tile] + e_half * ghalf[e_tile, 0]
    col = halfbase + pos_in_bucket // 128
    p_idx = pos_in_bucket % 128
    edst[e_core, p_idx, col] = e_local.astype(np.float32)
    # wrapped idx layout: edge i of its gather -> [i%16, gather_off*8 + i//16]
    # i = (pos within the half bucket); gather off in columns-of-8 = halfbase*8
    i_in_g = pos_in_bucket
    gidx[e_core, i_in_g % 16, halfbase * 8 + i_in_g // 16] = \
        e_tabidx.astype(np.int16)

    # per-core node-side arrays
    xT = np.zeros((N_CORES, N_FEAT, n_c), BF16_NP)
    dis_a = np.zeros((N_CORES, 128, T_tiles), np.float32)
    invdis_a = np.zeros((N_CORES, 1, n_c), BF16_NP)
    batchloc = np.full((N_CORES, 128, T_tiles), -1, np.float32)
    xT[core_of, :, slot] = x.astype(BF16_NP)        # fancy: (N, F) -> rows
    dis_a[core_of, slot % 128, slot // 128] = dis
    invdis_a[core_of, 0, slot] = invdis.astype(BF16_NP)
    batchloc[core_of, slot % 128, slot // 128] = (batch % GPW).astype(np.float32)

    # per-graph 1/max(count,1), [1, 256] per core (window-major)
    gcnt = np.bincount(batch, minlength=N_GRAPHS).astype(np.float32)
    rec = (1.0 / np.maximum(gcnt, 1.0)).reshape(N_CORES, 1, GPC)
    rec = np.ascontiguousarray(np.broadcast_to(rec, (N_CORES, 128, GPC)))

    # per-layer fp8 gain: z stored as G_l * z, undone in the epilogue
    disg = np.concatenate([dis_a * g for g in GAINS], axis=2)     # z-store
    disu = np.concatenate([dis_a / g for g in GAINS], axis=2)     # epilogue
    # per-(tile, half, chunk-of-8-Ktiles) valid-row counts, build order
    # must match the device loop exactly
    bucket_cnt = counts            # [N_CORES, T_tiles, 2]
    gcounts = []
    for t in range(T_tiles):
        for h in range(2):
            gn = int(ghalf[t, h])
            for g0 in range(0, gn, 8):
                gc_ = min(8, gn - g0)
                cval = np.clip(bucket_cnt[:, t, h] - g0 * 128, 0, gc_ * 128)
                gcounts.append(cval)
    gcounts = np.stack(gcounts, axis=1).astype(np.int32)   # [N_CORES, n_g]
    # every chunk needs >=1 valid idx (ucode/sim quirk); point it at row 0
    koff8 = []
    for t in range(T_tiles):
        for h in range(2):
            gn = int(ghalf[t, h])
            base = int(koff[t]) + (h * int(ghalf[t, 0]))
            for g0 in range(0, gn, 8):
                koff8.append(base + g0)
    for c in range(N_CORES):
        for g, cbase in enumerate(koff8):
            if gcounts[c, g] == 0:
                gidx[c, 0, cbase * 8] = 0
                gcounts[c, g] = 1
    tail = (counts.min(axis=0) // 128).astype(np.int64)       # [T,2]
    gmax = counts.max(axis=0).astype(np.int64)                 # [T,2]
    cfg = dict(T_half=T_half, kt=tuple(int(v) for v in kt),
               ghalf=tuple((int(a), int(b)) for a, b in ghalf),
               tail=tuple((int(a), int(b)) for a, b in tail),
               gmax=tuple((int(a), int(b)) for a, b in gmax))
    gidx = np.tile(gidx, (1, 8, 1)) if gidx.shape[1] == 16 else gidx
    arrays = dict(xT=xT, disg=disg, disu=disu, invdis=invdis_a,
                  batchloc=batchloc, gidx=gidx, edst=edst, rec=rec)
    return cfg, arrays


def pack_weights(W1, b1, W2, b2, W3, b3, Wl, bl, Wo, bo):
    """Pack the (replicated) weights into device layouts."""
    def to16(a):
        return np.asarray(a, np.float32).astype(BF16_NP)

    w1 = np.zeros((128, HIDDEN), BF16_NP)
    w1[:N_FEAT] = to16(W1)                                     # [64,256] pad K
    w2 = np.asarray(W2, np.float32).reshape(2, 128, HIDDEN)
    w2 = np.concatenate([to16(w2[0]), to16(w2[1])], axis=1)    # [128, 512]
    w3 = np.asarray(W3, np.float32).reshape(2, 128, HIDDEN)
    w3 = np.concatenate([to16(w3[0]), to16(w3[1])], axis=1)
    wl = np.asarray(Wl, np.float32).reshape(2, 128, 128)
    wl = np.concatenate([to16(wl[0]), to16(wl[1])], axis=1)    # [128, 256]
    wo = to16(np.asarray(Wo, np.float32).reshape(128, 1))      # [128, 1]
    b123 = np.stack([np.asarray(b, np.float32) * g
                     for b, g in zip((b1, b2, b3), GAINS)])
    b123 = b123.astype(BF16_NP).reshape(1, 3 * HIDDEN)         # [1, 768]
    bl_a = np.asarray(bl, np.float32).reshape(128, 1)          # [128, 1]
    bo_a = np.asarray(bo, np.float32).reshape(1, 1)
    return dict(w1=w1, w2=w2, w3=w3, wl=wl, wo=wo, b123=b123,
                bl=bl_a, bo=bo_a)


# -------------------------------------------------------------- bass kernel --

def build(cfg, debug_taps=False):
    """Build the SPMD Bass program (same graph on all 8 cores)."""
    T_half = cfg["T_half"]
    kt = cfg["kt"]
    ghalf = cfg["ghalf"]
    tail = cfg["tail"]
    gmax = cfg["gmax"]
    T_tiles = 2 * T_half
    n_c = T_tiles * 128
    t_kt = int(sum(kt))
    n_rows = N_CORES * n_c
    nh_rows = n_rows // 2

    nc = bacc.Bacc(None, target_bir_lowering=False,
                   dynamic_dma_scratch_size=49152,
                   num_swdge_queues=4)
    P = {}
    P["xT"] = nc.declare_dram_parameter("xT", [N_FEAT, n_c], BF16, False)
    P["disg"] = nc.declare_dram_parameter("disg", [128, 3 * T_tiles], F32, False)
    P["disu"] = nc.declare_dram_parameter("disu", [128, 3 * T_tiles], F32, False)
    P["invdis"] = nc.declare_dram_parameter("invdis", [1, n_c], BF16, False)
    P["batchloc"] = nc.declare_dram_parameter("batchloc", [128, T_tiles], F32, False)
    P["gidx"] = nc.declare_dram_parameter("gidx", [128, t_kt * 8], I16, False)
    P["edst"] = nc.declare_dram_parameter("edst", [128, t_kt], F32, False)
    P["rec"] = nc.declare_dram_parameter("rec", [128, GPC], F32, False)
    P["w1"] = nc.declare_dram_parameter("w1", [128, HIDDEN], BF16, False)
    P["w2"] = nc.declare_dram_parameter("w2", [128, 2 * HIDDEN], BF16, False)
    P["w3"] = nc.declare_dram_parameter("w3", [128, 2 * HIDDEN], BF16, False)
    P["wl"] = nc.declare_dram_parameter("wl", [128, 256], BF16, False)
    P["wo"] = nc.declare_dram_parameter("wo", [128, 1], BF16, False)
    P["b123"] = nc.declare_dram_parameter("b123", [1, 3 * HIDDEN], BF16, False)
    P["bl"] = nc.declare_dram_parameter("bl", [128, 1], F32, False)
    P["bo"] = nc.declare_dram_parameter("bo", [1, 1], F32, False)
    out_ext = nc.declare_dram_parameter("out", [1, GPC], F32, True)
    if debug_taps:
        dbg_z = nc.declare_dram_parameter("dbg_z", [n_c, HIDDEN], GDT, True)
        dbg_g = nc.declare_dram_parameter("dbg_g", [128, kt[0] * HIDDEN], GDT, True)
        dbg_h = nc.declare_dram_parameter("dbg_h", [128, HIDDEN], BF16, True)
        dbg_sel = nc.declare_dram_parameter("dbg_sel", [128, 128], GDT, True)

    rg = [list(range(N_CORES))]
    AF = mybir.ActivationFunctionType
    OP = mybir.AluOpType

    with tile.TileContext(nc) as tc:
        with (
            tc.tile_pool(name="const", bufs=1) as cpool,
            tc.tile_pool(name="big", bufs=1) as bigpool,
            tc.tile_pool(name="work", bufs=3) as wpool,
            tc.tile_pool(name="gath", bufs=3) as gpool,
            tc.tile_pool(name="sel", bufs=4) as spool,
            tc.tile_pool(name="ps", bufs=2, space="PSUM") as pspool,
            tc.tile_pool(name="psa", bufs=2, space="PSUM") as papool,
            tc.tile_pool(name="pspool", bufs=1, space="PSUM") as ppool,
            tc.tile_pool(name="dram", bufs=2, space="DRAM") as dpool,
        ):
            # ---- constants / parameter loads (once) ----
            iota_t = cpool.tile([128, 128], F32)
            nc.gpsimd.iota(iota_t[:], pattern=[[1, 128]], base=0,
                           channel_multiplier=0,
                           allow_small_or_imprecise_dtypes=True)
            ident = cpool.tile([128, 128], BF16)
            make_identity(nc, ident[:])
            ident8 = cpool.tile([128, 128], GDT)
            nc.vector.tensor_copy(ident8[:], ident[:])

            def load(name, shape, dt):
                t = cpool.tile(list(shape), dt, name=f"sb_{name}")
                nc.sync.dma_start(out=t[:], in_=P[name][:, :])
                return t

            xT_sb = bigpool.tile([N_FEAT, n_c], BF16)
            nc.sync.dma_start(out=xT_sb[:], in_=P["xT"][:, :])
            disg_sb = load("disg", (128, 3 * T_tiles), F32)
            disu_sb = load("disu", (128, 3 * T_tiles), F32)
            invdis_sb = load("invdis", (1, n_c), BF16)
            batchloc_sb = load("batchloc", (128, T_tiles), F32)
            gidx_sb = bigpool.tile([128, t_kt * 8], I16)
            nc.sync.dma_start(out=gidx_sb[:], in_=P["gidx"][:, :])
            gather_no = [0]
            edst_sb = bigpool.tile([128, t_kt], F32)
            nc.sync.dma_start(out=edst_sb[:], in_=P["edst"][:, :])
            rec_sb = load("rec", (128, GPC), F32)
            w1_sb = load("w1", (128, HIDDEN), BF16)
            w2_sb = load("w2", (128, 2 * HIDDEN), BF16)
            w3_sb = load("w3", (128, 2 * HIDDEN), BF16)
            wl_sb = load("wl", (128, 256), BF16)
            wo_sb = load("wo", (128, 1), BF16)
            b123_sb = load("b123", (1, 3 * HIDDEN), BF16)
            bl_sb = load("bl", (128, 1), F32)
            bo_sb = load("bo", (1, 1), F32)

            # persistent transposed activations for the dense matmuls
            hT0 = bigpool.tile([128, n_c], BF16)
            hT1 = bigpool.tile([128, n_c], BF16)

            pool_ps = [None, None]      # [chunk] psum tiles for poolT, per win
            out_sb = cpool.tile([1, GPC], F32)

            for layer in range(3):
                w_sb = (w1_sb, w2_sb, w3_sb)[layer]
                # -------- dense: z = dis * (h @ W), bf16, to DRAM shard ------
                zloc_e = dpool.tile([n_c // 2, HIDDEN], GDT, tag="zloc_e")
                zloc_o = dpool.tile([n_c // 2, HIDDEN], GDT, tag="zloc_o")
                for t in range(T_tiles):
                    sl = slice(t * 128, (t + 1) * 128)
                    psz = pspool.tile([128, HIDDEN], F32, tag="mm")
                    if layer == 0:
                        nc.tensor.matmul(psz[:], lhsT=xT_sb[:, sl],
                                         rhs=w_sb[:N_FEAT, :HIDDEN],
                                         start=True, stop=True)
                    else:
                        nc.tensor.matmul(psz[:], lhsT=hT0[:, sl],
                                         rhs=w_sb[:, 0:HIDDEN],
                                         start=True, stop=False)
                        nc.tensor.matmul(psz[:], lhsT=hT1[:, sl],
                                         rhs=w_sb[:, HIDDEN:2 * HIDDEN],
                                         start=False, stop=True)
                    zs = wpool.tile([128, HIDDEN], GDT, tag="zs")
                    nc.scalar.activation(
                        zs[:], psz[:], AF.Copy,
                        scale=disg_sb[:, layer * T_tiles + t:
                                      layer * T_tiles + t + 1])
                    hsl = slice(t * 64, (t + 1) * 64)
                    nc.sync.dma_start(out=zloc_e[hsl, :], in_=zs[0:64, :])
                    nc.sync.dma_start(out=zloc_o[hsl, :], in_=zs[64:128, :])

                # -------- AllGather the scaled z table ----------------------
                zfull_e = dpool.tile([nh_rows, HIDDEN], GDT, tag="zfull_e",
                                     addr_space="Shared")
                zfull_o = dpool.tile([nh_rows, HIDDEN], GDT, tag="zfull_o",
                                     addr_space="Shared")
                nc.gpsimd.collective_compute(
                    "AllGather", OP.bypass, replica_groups=rg,
                    ins=[zloc_e.opt()], outs=[zfull_e.opt()])
                nc.gpsimd.collective_compute(
                    "AllGather", OP.bypass, replica_groups=rg,
                    ins=[zloc_o.opt()], outs=[zfull_o.opt()])
                if debug_taps and layer == 0:
                    nc.gpsimd.dma_start(out=dbg_z[:n_c // 2, :],
                                        in_=zloc_e[:, :])
                    nc.gpsimd.dma_start(out=dbg_z[n_c // 2:, :],
                                        in_=zloc_o[:, :])

                # -------- per dst-tile: gather + one-hot segsum -------------
                gather_no[0] = 0
                b_row = b123_sb[0:1, layer * HIDDEN:(layer + 1) * HIDDEN]
                for t in range(T_tiles):
                    sl = slice(t * 128, (t + 1) * 128)
                    ktt = kt[t]
                    ko = int(sum(kt[:t]))
                    psa = papool.tile([128, HIDDEN], F32, tag="psa")
                    # bias as outer product (1/dis)[dst] x b  -> exact norm
                    nc.tensor.matmul(
                        psa[:], lhsT=invdis_sb[0:1, sl], rhs=b_row,
                        start=True, stop=False)
                    # self-loop term: agg[v] += z[v], from the local z store
                    hsl2 = slice(t * 64, (t + 1) * 64)
                    zsl = wpool.tile([128, HIDDEN], GDT, tag="zsl")
                    nc.sync.dma_start(out=zsl[0:64, :], in_=zloc_e[hsl2, :])
                    nc.sync.dma_start(out=zsl[64:128, :], in_=zloc_o[hsl2, :])
                    nc.tensor.matmul(
                        psa[:], lhsT=ident8[:], rhs=zsl[:],
                        start=False, stop=(ktt == 0))
                    if ktt > 0:
                        ge, go = ghalf[t]
                        gath = gpool.tile([128, ktt * HIDDEN], GDT,
                                          tag="gath")
                        for (gn, tab, c0, gm) in (
                                (ge, zfull_e, 0, gmax[t][0]),
                                (go, zfull_o, ge, gmax[t][1])):
                            if gn > 0 and gm < gn * 128:
                                # rows past the max count stay unwritten in
                                # the last K-tile; zero them so sel=0 never
                                # multiplies NaN bit patterns
                                nc.vector.memset(
                                    gath[:, (c0 + gn - 1) * HIDDEN:
                                         (c0 + gn) * HIDDEN], 0.0)
                            # <=8 K-tiles (1024 rows) per instruction so one
                            # gather's descriptors fit the SWDGE ring; the
                            # last chunk gathers only the max-over-cores
                            # row count instead of the 128-rounded count
                            for g0 in range(0, gn, 8):
                                gc = min(8, gn - g0)
                                nidx = min(gc * 128, int(gm) - g0 * 128)
                                cc = c0 + g0
                                b8 = (ko + cc) * 8
                                gv = gath[:, cc * HIDDEN:
                                          (cc + gc) * HIDDEN] \
                                    .rearrange("p (k h) -> p k h", h=HIDDEN)
                                nc.gpsimd.dma_gather(
                                    out_ap=gv, in_ap=tab[:, :],
                                    idxs_ap=gidx_sb[:, b8:
                                                    b8 + (nidx + 15) // 16],
                                    num_idxs=nidx, num_idxs_reg=nidx,
                                    elem_size=HIDDEN,
                                    queue_num=gather_no[0] % 4)
                                gather_no[0] += 1
                        if debug_taps and layer == 0 and t == 0:
                            nc.gpsimd.dma_start(out=dbg_g[:, :], in_=gath[:])
                        for j in range(ktt):
                            sel = spool.tile([128, 128], GDT, tag="sel")
                            nc.vector.tensor_tensor(
                                sel[:],
                                edst_sb[:, ko + j:ko + j + 1]
                                .to_broadcast([128, 128]),
                                iota_t[:], op=OP.is_equal)
                            if debug_taps and layer == 0 and t == 0 and j == 0:
                                nc.sync.dma_start(out=dbg_sel[:, :], in_=sel[:])
                            nc.tensor.matmul(
                                psa[:], lhsT=sel[:],
                                rhs=gath[:, j * HIDDEN:(j + 1) * HIDDEN],
                                start=False, stop=(j == ktt - 1))
                    # h = relu(dis * agg + b)
                    h_sb = wpool.tile([128, HIDDEN], BF16, tag="h")
                    nc.scalar.activation(
                        h_sb[:], psa[:], AF.Relu,
                        scale=disu_sb[:, layer * T_tiles + t:
                                      layer * T_tiles + t + 1])
                    if debug_taps and layer == 0 and t == 0:
                        nc.sync.dma_start(out=dbg_h[:, :], in_=h_sb[:])
                    if layer < 2:
                        for cch in range(2):
                            pst = pspool.tile([128, 128], BF16, tag="mm")
                            nc.tensor.transpose(
                                pst[:], h_sb[:, cch * 128:(cch + 1) * 128],
                                ident[:])
                            hT = (hT0, hT1)[cch]
                            nc.vector.tensor_copy(hT[:, sl], pst[:])
                    else:
                        win = t // T_half
                        first = (t % T_half) == 0
                        last = (t % T_half) == T_half - 1
                        if first:
                            pool_ps[0] = ppool.tile([128, 128], F32, name="poolT0",
                                                    tag="poolT0", bufs=1)
                            pool_ps[1] = ppool.tile([128, 128], F32, name="poolT1",
                                                    tag="poolT1", bufs=1)
                        selp = spool.tile([128, 128], BF16, tag="selp")
                        nc.vector.tensor_tensor(
                            selp[:],
                            batchloc_sb[:, t:t + 1].to_broadcast([128, 128]),
                            iota_t[:], op=OP.is_equal)
                        for cch in range(2):
                            nc.tensor.matmul(
                                pool_ps[cch][:],
                                lhsT=h_sb[:, cch * 128:(cch + 1) * 128],
                                rhs=selp[:], start=first, stop=last)
                        if last:
                            # ---- head for this window of 128 graphs ----
                            rrow = rec_sb[:, win * GPW:(win + 1) * GPW]
                            psu = papool.tile([128, GPW], F32, tag="head", bufs=1)
                            for cch in range(2):
                                gT = wpool.tile([128, GPW], BF16,
                                                tag="gT")
                                nc.vector.tensor_tensor(
                                    gT[:], pool_ps[cch][:, :GPW],
                                    rrow, op=OP.mult)
                                nc.tensor.matmul(
                                    psu[:],
                                    lhsT=wl_sb[:, cch * 128:(cch + 1) * 128],
                                    rhs=gT[:], start=(cch == 0),
                                    stop=(cch == 1))
                            uT = wpool.tile([128, GPW], BF16, tag="uT")
                            nc.scalar.activation(uT[:], psu[:], AF.Relu,
                                                 bias=bl_sb[:, 0:1])
                            pso = papool.tile([1, GPW], F32, tag="head", bufs=1)
                            nc.tensor.matmul(pso[:], lhsT=wo_sb[:, 0:1],
                                             rhs=uT[:], start=True, stop=True)
                            nc.vector.tensor_scalar(
                                out_sb[0:1, win * GPW:(win + 1) * GPW],
                                pso[:], bo_sb[0:1, 0:1], None, op0=OP.add)
            nc.sync.dma_start(out=out_ext[:, :], in_=out_sb[:])
    nc.finalize()
    return nc


# ------------------------------------------------------------------ runner --

_CACHE = {}


def _get_program(cfg):
    key = (cfg["T_half"], cfg["kt"])
    if key not in _CACHE:
        _CACHE[key] = build(cfg)
    return _CACHE[key]


def kernel(x, edge_index, batch, W1, b1, W2, b2, W3, b3, Wl, bl, Wo, bo):
    from concourse.bass_utils import run_bass_kernel_spmd

    cfg, arrays = preprocess(x, edge_index, batch)
    wts = pack_weights(W1, b1, W2, b2, W3, b3, Wl, bl, Wo, bo)
    nc = _get_program(cfg)

    in_maps = []
    for c in range(N_CORES):
        m = {k: np.ascontiguousarray(v[c]) for k, v in arrays.items()}
        m.update(wts)
        in_maps.append(m)

    res = run_bass_kernel_spmd(nc, in_maps, core_ids=list(range(N_CORES)))
    outs = res.results
    out = np.concatenate([outs[c]["out"].reshape(GPC) for c in range(N_CORES)])
    return out.reshape(N_GRAPHS, 1).astype(np.float32)

